# revision 11
# baseline (speedup 1.0000x reference)
"""Trainium2 Bass kernel for a custom LSTM cell.

reference:
    z = concat([h_tm1, inputs], -1) @ kernel      # [B, 4U]
    i, f, g, o = split(z, 4, -1)
    c = sigmoid(f) * c_tm1 + sigmoid(i) * tanh(g)
    h = sigmoid(o) * tanh(c)
    returns (h, c)

Sharding over 8 NeuronCores: 2-way over batch x 4-way over units
(each gate's block co-located per core).  Per core:
    z_blk = A_half @ W[:, 4 gate slices of 256] in bf16 (full PE rate,
    half the HBM traffic of fp32r), gate math on-chip, outputs
    [1024, 256] h/c blocks in fp32.  Host only slices/casts/concats.

Schedule (per core):
  DMA: few LARGE transfers (ring cost ~0.65us each, serialized per
       queue): at on the Sync HWDGE ring, weights on the Scalar ring
       (wk_hi strictly behind wk_lo in the FIFO), ct on GpSimd.
       Chunks sized so arrival tracks PE consumption; sub-3us worst-case
       stalls so HAM never drops back to K=4/8 mid-stream.
  Warm-up: ~20 dummy matmuls on an uninitialized raw SBUF tile, no data
       deps, so the PE burns its 0.65->2.4GHz clock ramp and the HAM
       K=4/8 window while the first input chunks are still in flight.
  phase 1: all m, i|f columns, k-outer round-robin over 8 PSUM banks;
       each closes with Sigmoid -> sig_if (bf16), freeing its bank.
  phase 2: per-m serial g|o accumulation + epilogue, c|h packed in one
       [128,512] tile -> one output DMA per m on the idle Sync ring.
       Last m is split g-half/o-half to shorten the final serial chain.
"""

import sys

sys.path.insert(0, "/opt/trn_rl_repo")

import ml_dtypes
import numpy as np

BF16 = ml_dtypes.bfloat16

BATCH, INPUT_DIM, UNITS = 2048, 512, 1024
K = UNITS + INPUT_DIM  # contraction dim, 1536
R, C = 2, 4  # batch halves x unit quarters
BR = BATCH // R  # 1024 batch rows per core
UC = UNITS // C  # 256 units per core
KS = K // 128  # 12 k-subtiles
M = BR // 128  # 8 batch sub-chunks per core

_CACHE = {}


def _build_nc():
    import concourse.tile as tile
    from concourse import bacc, mybir

    f32 = mybir.dt.float32
    bf16 = mybir.dt.bfloat16
    Sig = mybir.ActivationFunctionType.Sigmoid
    Tanh = mybir.ActivationFunctionType.Tanh

    nc = bacc.Bacc("TRN2")
    at_in = nc.declare_dram_parameter("at", [K, BR], bf16, isOutput=False)
    wklo_in = nc.declare_dram_parameter("wk_lo", [K, 512], bf16, isOutput=False)
    wkhi_in = nc.declare_dram_parameter("wk_hi", [K, 512], bf16, isOutput=False)
    ct_in = nc.declare_dram_parameter("ct", [BR, UC], bf16, isOutput=False)
    ch_out = nc.declare_dram_parameter("ch_out", [BR, 512], bf16, isOutput=True)

    # Raw (non-pool) scratch for warm-up matmuls: contents irrelevant,
    # no writer -> no dependencies -> earliest possible PE start.
    dummy = nc.alloc_sbuf_tensor("warm", [128, 512], bf16)

    with tile.TileContext(nc) as tc:
        with (
            tc.tile_pool(name="sb", bufs=1) as sb,
            tc.tile_pool(name="psum", bufs=8, space="PSUM") as psum,
        ):
            at = sb.tile([128, KS, BR], bf16)
            wk_lo = sb.tile([128, KS, 512], bf16)  # i|f columns
            wk_hi = sb.tile([128, KS, 512], bf16)  # g|o columns
            ct = sb.tile([128, M, UC], bf16)
            sig_if = sb.tile([128, M, 512], bf16)
            fc_all = sb.tile([128, M, UC], bf16)

            at_r = at_in[:].rearrange("(ko p) n -> p ko n", p=128)
            wklo_r = wklo_in[:].rearrange("(ko p) n -> p ko n", p=128)
            wkhi_r = wkhi_in[:].rearrange("(ko p) n -> p ko n", p=128)
            ct_r = ct_in[:].rearrange("(m p) u -> p m u", p=128)

            # at on the Sync HWDGE ring, consumption-ordered.
            nc.sync.dma_start(at[:, 0:1, 0:512], at_r[:, 0:1, 0:512])
            nc.sync.dma_start(at[:, 0:1, 512:1024], at_r[:, 0:1, 512:1024])
            for ks in (
                slice(1, 2),
                slice(2, 4),
                slice(4, 6),
                slice(6, 8),
                slice(8, 10),
                slice(10, KS),
            ):
                nc.sync.dma_start(at[:, ks, :], at_r[:, ks, :])
            # weights on the Scalar HWDGE ring; wk_hi strictly behind wk_lo.
            for ks in (slice(0, 1), slice(1, 3), slice(3, 6), slice(6, 9), slice(9, KS)):
                nc.scalar.dma_start(wk_lo[:, ks, :], wklo_r[:, ks, :])
            for ks in (slice(0, 4), slice(4, 8), slice(8, KS)):
                nc.scalar.dma_start(wk_hi[:, ks, :], wkhi_r[:, ks, :])
            # ct on the GpSimd SWDGE queue.
            nc.gpsimd.dma_start(ct[:, :, :], ct_r[:, :, :])

            # phase 1: all m, i|f columns, k-outer round-robin
            plo = [
                psum.tile([128, 512], f32, tag="ps", name=f"plo{m}") for m in range(M)
            ]
            for _ in range(17):
                nc.tensor.matmul(
                    plo[0][:],
                    dummy[:, 0:128],
                    dummy[:],
                    start=True,
                    stop=True,
                    skip_group_check=True,
                )
            for k in range(KS):
                for m in range(M):
                    nc.tensor.matmul(
                        plo[m][:],
                        at[:, k, m * 128 : (m + 1) * 128],
                        wk_lo[:, k, :],
                        start=(k == 0),
                        stop=(k == KS - 1),
                    )
            for m in range(M):
                nc.scalar.activation(sig_if[:, m, :], plo[m][:], Sig)
            # f * c_tm1 off the epilogue critical path (DVE is idle here)
            for m in range(M):
                nc.vector.tensor_mul(
                    fc_all[:, m, :], sig_if[:, m, UC : 2 * UC], ct[:, m, :]
                )

            # phase 2: per-m serial g|o accumulation + epilogue
            for m in range(M):
                ms = slice(m * 128, (m + 1) * 128)
                last = m == M - 1
                och = sb.tile([128, 512], bf16, tag="och", bufs=3)
                if not last:
                    phi = psum.tile([128, 512], f32, tag="ps", name=f"phi{m}")
                    for k in range(KS):
                        nc.tensor.matmul(
                            phi[:],
                            at[:, k, ms],
                            wk_hi[:, k, :],
                            start=(k == 0),
                            stop=(k == KS - 1),
                        )
                    phi_g, phi_o = phi[:, 0:UC], phi[:, UC : 2 * UC]
                else:
                    # split the last m so tanh(g)/i*g/c/tanh(c) overlap the
                    # o-half matmuls, shortening the end-of-kernel chain.
                    pg = psum.tile([128, UC], f32, tag="ps", name="pg")
                    po = psum.tile([128, UC], f32, tag="ps", name="po")
                    for k in range(KS):
                        nc.tensor.matmul(
                            pg[:],
                            at[:, k, ms],
                            wk_hi[:, k, 0:UC],
                            start=(k == 0),
                            stop=(k == KS - 1),
                        )
                    for k in range(KS):
                        nc.tensor.matmul(
                            po[:],
                            at[:, k, ms],
                            wk_hi[:, k, UC : 2 * UC],
                            start=(k == 0),
                            stop=(k == KS - 1),
                        )
                    phi_g, phi_o = pg[:], po[:]
                tg = sb.tile([128, UC], bf16, tag="tg", bufs=3)
                nc.scalar.activation(tg[:], phi_g, Tanh)
                ig = sb.tile([128, UC], bf16, tag="ig", bufs=3)
                nc.vector.tensor_mul(ig[:], sig_if[:, m, 0:UC], tg[:])
                nc.vector.tensor_add(och[:, 0:UC], fc_all[:, m, :], ig[:])
                th = sb.tile([128, UC], bf16, tag="th", bufs=3)
                nc.scalar.activation(th[:], och[:, 0:UC], Tanh)
                so = sb.tile([128, UC], bf16, tag="so", bufs=3)
                nc.scalar.activation(so[:], phi_o, Sig)
                nc.vector.tensor_mul(och[:, UC : 2 * UC], so[:], th[:])
                # Sync HWDGE ring is idle during phase 2.
                if last:
                    nc.sync.dma_start(ch_out[ms, 0:UC], och[:, 0:UC])
                    nc.sync.dma_start(ch_out[ms, UC : 2 * UC], och[:, UC : 2 * UC])
                else:
                    nc.sync.dma_start(ch_out[ms, :], och[:])

    nc.compile()
    return nc


def get_nc():
    if "nc" not in _CACHE:
        _CACHE["nc"] = _build_nc()
    return _CACHE["nc"]


def make_in_maps(inputs, h_tm1, c_tm1, kernel):
    x = np.asarray(inputs, dtype=np.float32)
    h = np.asarray(h_tm1, dtype=np.float32)
    c = np.asarray(c_tm1, dtype=np.float32).astype(BF16)
    w = np.asarray(kernel, dtype=np.float32).astype(BF16)
    at_full = np.ascontiguousarray(
        np.concatenate([h, x], axis=1).T.astype(BF16)
    )  # [K, B] bf16
    in_maps = []
    for core in range(R * C):
        r, ci = divmod(core, C)
        at_np = np.ascontiguousarray(at_full[:, r * BR : (r + 1) * BR])
        gates = [
            w[:, g * UNITS + ci * UC : g * UNITS + (ci + 1) * UC] for g in range(4)
        ]
        wklo_np = np.ascontiguousarray(np.concatenate(gates[0:2], axis=1))
        wkhi_np = np.ascontiguousarray(np.concatenate(gates[2:4], axis=1))
        ct_np = np.ascontiguousarray(c[r * BR : (r + 1) * BR, ci * UC : (ci + 1) * UC])
        in_maps.append(
            {"at": at_np, "wk_lo": wklo_np, "wk_hi": wkhi_np, "ct": ct_np}
        )
    return in_maps


def assemble(results):
    h_new = np.empty((BATCH, UNITS), dtype=np.float32)
    c_new = np.empty((BATCH, UNITS), dtype=np.float32)
    for core in range(R * C):
        r, ci = divmod(core, C)
        ch = results[core]["ch_out"].astype(np.float32)
        c_new[r * BR : (r + 1) * BR, ci * UC : (ci + 1) * UC] = ch[:, 0:UC]
        h_new[r * BR : (r + 1) * BR, ci * UC : (ci + 1) * UC] = ch[:, UC : 2 * UC]
    return h_new, c_new


def kernel(inputs, h_tm1, c_tm1, kernel):
    from concourse.bass_utils import run_bass_kernel_spmd

    nc = get_nc()
    in_maps = make_in_maps(inputs, h_tm1, c_tm1, kernel)
    res = run_bass_kernel_spmd(nc, in_maps, list(range(R * C)), trace=False)
    return assemble(res.results)


# revision 12
# speedup vs baseline: 1.0394x; 1.0394x over previous
"""Trainium2 Bass kernel for a custom LSTM cell.

reference:
    z = concat([h_tm1, inputs], -1) @ kernel      # [B, 4U]
    i, f, g, o = split(z, 4, -1)
    c = sigmoid(f) * c_tm1 + sigmoid(i) * tanh(g)
    h = sigmoid(o) * tanh(c)
    returns (h, c)

Sharding over 8 NeuronCores: 2-way over batch x 4-way over units
(each gate's block co-located per core).  Per core:
    z_blk = A_half @ W[:, 4 gate slices of 256] in bf16 (full PE rate,
    half the HBM traffic of fp32r), gate math on-chip, outputs
    [1024, 256] h/c blocks in fp32.  Host only slices/casts/concats.

Schedule (per core):
  DMA: few LARGE transfers (ring cost ~0.65us each, serialized per
       queue): at on the Sync HWDGE ring, weights on the Scalar ring
       (wk_hi strictly behind wk_lo in the FIFO), ct on GpSimd.
       Chunks sized so arrival tracks PE consumption; sub-3us worst-case
       stalls so HAM never drops back to K=4/8 mid-stream.
  Warm-up: ~20 dummy matmuls on an uninitialized raw SBUF tile, no data
       deps, so the PE burns its 0.65->2.4GHz clock ramp and the HAM
       K=4/8 window while the first input chunks are still in flight.
  phase 1: all m, i|f columns, k-outer round-robin over 8 PSUM banks;
       each closes with Sigmoid -> sig_if (bf16), freeing its bank.
  phase 2: per-m serial g|o accumulation + epilogue, c|h packed in one
       [128,512] tile -> one output DMA per m on the idle Sync ring.
       Last m is split g-half/o-half to shorten the final serial chain.
"""

import sys

sys.path.insert(0, "/opt/trn_rl_repo")

import ml_dtypes
import numpy as np

BF16 = ml_dtypes.bfloat16

BATCH, INPUT_DIM, UNITS = 2048, 512, 1024
K = UNITS + INPUT_DIM  # contraction dim, 1536
R, C = 2, 4  # batch halves x unit quarters
BR = BATCH // R  # 1024 batch rows per core
UC = UNITS // C  # 256 units per core
KS = K // 128  # 12 k-subtiles
M = BR // 128  # 8 batch sub-chunks per core

_CACHE = {}


def _build_nc():
    import concourse.tile as tile
    from concourse import bacc, mybir

    f32 = mybir.dt.float32
    bf16 = mybir.dt.bfloat16
    Sig = mybir.ActivationFunctionType.Sigmoid
    Tanh = mybir.ActivationFunctionType.Tanh

    nc = bacc.Bacc("TRN2")
    at_in = nc.declare_dram_parameter("at", [K, BR], bf16, isOutput=False)
    wklo_in = nc.declare_dram_parameter("wk_lo", [K, 512], bf16, isOutput=False)
    wkhi_in = nc.declare_dram_parameter("wk_hi", [K, 512], bf16, isOutput=False)
    ct_in = nc.declare_dram_parameter("ct", [BR, UC], bf16, isOutput=False)
    ch_out = nc.declare_dram_parameter("ch_out", [BR, 512], bf16, isOutput=True)

    # Raw (non-pool) scratch for warm-up matmuls: contents irrelevant,
    # no writer -> no dependencies -> earliest possible PE start.
    dummy = nc.alloc_sbuf_tensor("warm", [128, 512], bf16)

    with tile.TileContext(nc) as tc:
        with (
            tc.tile_pool(name="sb", bufs=1) as sb,
            tc.tile_pool(name="psum", bufs=8, space="PSUM") as psum,
        ):
            at = sb.tile([128, KS, BR], bf16)
            wk_lo = sb.tile([128, KS, 512], bf16)  # i|f columns
            wk_hi = sb.tile([128, KS, 512], bf16)  # g|o columns
            ct = sb.tile([128, M, UC], bf16)
            sig_if = sb.tile([128, M, 512], bf16)
            fc_all = sb.tile([128, M, UC], bf16)

            at_r = at_in[:].rearrange("(ko p) n -> p ko n", p=128)
            wklo_r = wklo_in[:].rearrange("(ko p) n -> p ko n", p=128)
            wkhi_r = wkhi_in[:].rearrange("(ko p) n -> p ko n", p=128)
            ct_r = ct_in[:].rearrange("(m p) u -> p m u", p=128)

            # at on the Sync HWDGE ring, consumption-ordered.  Per-k chunks:
            # the 256KB transfer time dominates the ~0.65us ring cost, and
            # fine granularity turns a slow-core data lag into many sub-1us
            # PE waits (HAM-safe) instead of one >3us stall (HAM drop).
            nc.sync.dma_start(at[:, 0:1, 0:512], at_r[:, 0:1, 0:512])
            nc.sync.dma_start(at[:, 0:1, 512:1024], at_r[:, 0:1, 512:1024])
            for j in range(1, KS):
                nc.sync.dma_start(at[:, j : j + 1, :], at_r[:, j : j + 1, :])
            # weights on the Scalar HWDGE ring; wk_hi strictly behind wk_lo.
            for ks in (
                slice(0, 1),
                slice(1, 2),
                slice(2, 4),
                slice(4, 6),
                slice(6, 8),
                slice(8, 10),
                slice(10, KS),
            ):
                nc.scalar.dma_start(wk_lo[:, ks, :], wklo_r[:, ks, :])
            for ks in (slice(0, 4), slice(4, 8), slice(8, KS)):
                nc.scalar.dma_start(wk_hi[:, ks, :], wkhi_r[:, ks, :])
            # ct on the GpSimd SWDGE queue.
            nc.gpsimd.dma_start(ct[:, :, :], ct_r[:, :, :])

            # phase 1: all m, i|f columns, k-outer round-robin
            plo = [
                psum.tile([128, 512], f32, tag="ps", name=f"plo{m}") for m in range(M)
            ]
            for _ in range(17):
                nc.tensor.matmul(
                    plo[0][:],
                    dummy[:, 0:128],
                    dummy[:],
                    start=True,
                    stop=True,
                    skip_group_check=True,
                )
            for k in range(KS):
                for m in range(M):
                    nc.tensor.matmul(
                        plo[m][:],
                        at[:, k, m * 128 : (m + 1) * 128],
                        wk_lo[:, k, :],
                        start=(k == 0),
                        stop=(k == KS - 1),
                    )
            for m in range(M):
                nc.scalar.activation(sig_if[:, m, :], plo[m][:], Sig)
            # f * c_tm1 off the epilogue critical path (DVE is idle here)
            for m in range(M):
                nc.vector.tensor_mul(
                    fc_all[:, m, :], sig_if[:, m, UC : 2 * UC], ct[:, m, :]
                )

            # phase 2: per-m serial g|o accumulation + epilogue
            for m in range(M):
                ms = slice(m * 128, (m + 1) * 128)
                last = m == M - 1
                och = sb.tile([128, 512], bf16, tag="och", bufs=3)
                if not last:
                    phi = psum.tile([128, 512], f32, tag="ps", name=f"phi{m}")
                    for k in range(KS):
                        nc.tensor.matmul(
                            phi[:],
                            at[:, k, ms],
                            wk_hi[:, k, :],
                            start=(k == 0),
                            stop=(k == KS - 1),
                        )
                    phi_g, phi_o = phi[:, 0:UC], phi[:, UC : 2 * UC]
                else:
                    # split the last m so tanh(g)/i*g/c/tanh(c) overlap the
                    # o-half matmuls, shortening the end-of-kernel chain.
                    pg = psum.tile([128, UC], f32, tag="ps", name="pg")
                    po = psum.tile([128, UC], f32, tag="ps", name="po")
                    for k in range(KS):
                        nc.tensor.matmul(
                            pg[:],
                            at[:, k, ms],
                            wk_hi[:, k, 0:UC],
                            start=(k == 0),
                            stop=(k == KS - 1),
                        )
                    for k in range(KS):
                        nc.tensor.matmul(
                            po[:],
                            at[:, k, ms],
                            wk_hi[:, k, UC : 2 * UC],
                            start=(k == 0),
                            stop=(k == KS - 1),
                        )
                    phi_g, phi_o = pg[:], po[:]
                tg = sb.tile([128, UC], bf16, tag="tg", bufs=3)
                nc.scalar.activation(tg[:], phi_g, Tanh)
                ig = sb.tile([128, UC], bf16, tag="ig", bufs=3)
                nc.vector.tensor_mul(ig[:], sig_if[:, m, 0:UC], tg[:])
                nc.vector.tensor_add(och[:, 0:UC], fc_all[:, m, :], ig[:])
                th = sb.tile([128, UC], bf16, tag="th", bufs=3)
                nc.scalar.activation(th[:], och[:, 0:UC], Tanh)
                so = sb.tile([128, UC], bf16, tag="so", bufs=3)
                nc.scalar.activation(so[:], phi_o, Sig)
                nc.vector.tensor_mul(och[:, UC : 2 * UC], so[:], th[:])
                # Sync HWDGE ring is idle during phase 2.
                if last:
                    nc.sync.dma_start(ch_out[ms, 0:UC], och[:, 0:UC])
                    nc.sync.dma_start(ch_out[ms, UC : 2 * UC], och[:, UC : 2 * UC])
                else:
                    nc.sync.dma_start(ch_out[ms, :], och[:])

    nc.compile()
    return nc


def get_nc():
    if "nc" not in _CACHE:
        _CACHE["nc"] = _build_nc()
    return _CACHE["nc"]


def make_in_maps(inputs, h_tm1, c_tm1, kernel):
    x = np.asarray(inputs, dtype=np.float32)
    h = np.asarray(h_tm1, dtype=np.float32)
    c = np.asarray(c_tm1, dtype=np.float32).astype(BF16)
    w = np.asarray(kernel, dtype=np.float32).astype(BF16)
    at_full = np.ascontiguousarray(
        np.concatenate([h, x], axis=1).T.astype(BF16)
    )  # [K, B] bf16
    in_maps = []
    for core in range(R * C):
        r, ci = divmod(core, C)
        at_np = np.ascontiguousarray(at_full[:, r * BR : (r + 1) * BR])
        gates = [
            w[:, g * UNITS + ci * UC : g * UNITS + (ci + 1) * UC] for g in range(4)
        ]
        wklo_np = np.ascontiguousarray(np.concatenate(gates[0:2], axis=1))
        wkhi_np = np.ascontiguousarray(np.concatenate(gates[2:4], axis=1))
        ct_np = np.ascontiguousarray(c[r * BR : (r + 1) * BR, ci * UC : (ci + 1) * UC])
        in_maps.append(
            {"at": at_np, "wk_lo": wklo_np, "wk_hi": wkhi_np, "ct": ct_np}
        )
    return in_maps


def assemble(results):
    h_new = np.empty((BATCH, UNITS), dtype=np.float32)
    c_new = np.empty((BATCH, UNITS), dtype=np.float32)
    for core in range(R * C):
        r, ci = divmod(core, C)
        ch = results[core]["ch_out"].astype(np.float32)
        c_new[r * BR : (r + 1) * BR, ci * UC : (ci + 1) * UC] = ch[:, 0:UC]
        h_new[r * BR : (r + 1) * BR, ci * UC : (ci + 1) * UC] = ch[:, UC : 2 * UC]
    return h_new, c_new


def kernel(inputs, h_tm1, c_tm1, kernel):
    from concourse.bass_utils import run_bass_kernel_spmd

    nc = get_nc()
    in_maps = make_in_maps(inputs, h_tm1, c_tm1, kernel)
    res = run_bass_kernel_spmd(nc, in_maps, list(range(R * C)), trace=False)
    return assemble(res.results)


# revision 13
# speedup vs baseline: 1.1187x; 1.0763x over previous
"""Trainium2 Bass kernel for a custom LSTM cell.

reference:
    z = concat([h_tm1, inputs], -1) @ kernel      # [B, 4U]
    i, f, g, o = split(z, 4, -1)
    c = sigmoid(f) * c_tm1 + sigmoid(i) * tanh(g)
    h = sigmoid(o) * tanh(c)
    returns (h, c)

Sharding over 8 NeuronCores: 2-way over batch x 4-way over units
(each gate's block co-located per core).  Per core:
    z_blk = A_half @ W[:, 4 gate slices of 256] in bf16 (full PE rate,
    half the HBM traffic of fp32r), gate math on-chip, outputs
    [1024, 256] h/c blocks in fp32.  Host only slices/casts/concats.

Schedule (per core):
  DMA: few LARGE transfers (ring cost ~0.65us each, serialized per
       queue): at on the Sync HWDGE ring, weights on the Scalar ring
       (wk_hi strictly behind wk_lo in the FIFO), ct on GpSimd.
       Chunks sized so arrival tracks PE consumption; sub-3us worst-case
       stalls so HAM never drops back to K=4/8 mid-stream.
  Warm-up: ~20 dummy matmuls on an uninitialized raw SBUF tile, no data
       deps, so the PE burns its 0.65->2.4GHz clock ramp and the HAM
       K=4/8 window while the first input chunks are still in flight.
  phase 1: all m, i|f columns, k-outer round-robin over 8 PSUM banks;
       each closes with Sigmoid -> sig_if (bf16), freeing its bank.
  phase 2: per-m serial g|o accumulation + epilogue, c|h packed in one
       [128,512] tile -> one output DMA per m on the idle Sync ring.
       Last m is split g-half/o-half to shorten the final serial chain.
"""

import sys

sys.path.insert(0, "/opt/trn_rl_repo")

import ml_dtypes
import numpy as np

BF16 = ml_dtypes.bfloat16

BATCH, INPUT_DIM, UNITS = 2048, 512, 1024
K = UNITS + INPUT_DIM  # contraction dim, 1536
R, C = 2, 4  # batch halves x unit quarters
BR = BATCH // R  # 1024 batch rows per core
UC = UNITS // C  # 256 units per core
KS = K // 128  # 12 k-subtiles
M = BR // 128  # 8 batch sub-chunks per core

_CACHE = {}


def _build_nc():
    import concourse.tile as tile
    from concourse import bacc, mybir

    f32 = mybir.dt.float32
    bf16 = mybir.dt.bfloat16
    Sig = mybir.ActivationFunctionType.Sigmoid
    Tanh = mybir.ActivationFunctionType.Tanh

    nc = bacc.Bacc("TRN2")
    at_in = nc.declare_dram_parameter("at", [K, BR], bf16, isOutput=False)
    wklo_in = nc.declare_dram_parameter("wk_lo", [K, 512], bf16, isOutput=False)
    wkhi_in = nc.declare_dram_parameter("wk_hi", [K, 512], bf16, isOutput=False)
    ct_in = nc.declare_dram_parameter("ct", [BR, UC], bf16, isOutput=False)
    ch_out = nc.declare_dram_parameter("ch_out", [BR, 512], bf16, isOutput=True)

    # Raw (non-pool) scratch for warm-up matmuls: contents irrelevant,
    # no writer -> no dependencies -> earliest possible PE start.
    dummy = nc.alloc_sbuf_tensor("warm", [128, 512], bf16)

    with tile.TileContext(nc) as tc:
        with (
            tc.tile_pool(name="sb", bufs=1) as sb,
            tc.tile_pool(name="psum", bufs=8, space="PSUM") as psum,
        ):
            at = sb.tile([128, KS, BR], bf16)
            wk_lo = sb.tile([128, KS, 512], bf16)  # i|f columns
            wk_hi = sb.tile([128, KS, 512], bf16)  # g|o columns
            ct = sb.tile([128, M, UC], bf16)
            sig_if = sb.tile([128, M, 512], bf16)
            fc_all = sb.tile([128, M, UC], bf16)

            at_r = at_in[:].rearrange("(ko p) n -> p ko n", p=128)
            wklo_r = wklo_in[:].rearrange("(ko p) n -> p ko n", p=128)
            wkhi_r = wkhi_in[:].rearrange("(ko p) n -> p ko n", p=128)
            ct_r = ct_in[:].rearrange("(m p) u -> p m u", p=128)

            # at on the Sync HWDGE ring, consumption-ordered.  Per-k chunks:
            # the 256KB transfer time dominates the ~0.65us ring cost, and
            # fine granularity turns a slow-core data lag into many sub-1us
            # PE waits (HAM-safe) instead of one >3us stall (HAM drop).
            nc.sync.dma_start(at[:, 0:1, 0:512], at_r[:, 0:1, 0:512])
            nc.sync.dma_start(at[:, 0:1, 512:1024], at_r[:, 0:1, 512:1024])
            for j in range(1, KS):
                nc.sync.dma_start(at[:, j : j + 1, :], at_r[:, j : j + 1, :])
            # weights on the Scalar HWDGE ring; wk_hi strictly behind wk_lo.
            for ks in (
                slice(0, 1),
                slice(1, 2),
                slice(2, 4),
                slice(4, 6),
                slice(6, 8),
                slice(8, 10),
                slice(10, KS),
            ):
                nc.scalar.dma_start(wk_lo[:, ks, :], wklo_r[:, ks, :])
            for ks in (slice(0, 4), slice(4, 8), slice(8, KS)):
                nc.scalar.dma_start(wk_hi[:, ks, :], wkhi_r[:, ks, :])
            # ct on the GpSimd SWDGE queue.
            nc.gpsimd.dma_start(ct[:, :, :], ct_r[:, :, :])

            # phase 1: all m, i|f columns, k-outer round-robin
            plo = [
                psum.tile([128, 512], f32, tag="ps", name=f"plo{m}") for m in range(M)
            ]
            for _ in range(13):
                nc.tensor.matmul(
                    plo[0][:],
                    dummy[:, 0:128],
                    dummy[:],
                    start=True,
                    stop=True,
                    skip_group_check=True,
                )
            for k in range(KS):
                for m in range(M):
                    nc.tensor.matmul(
                        plo[m][:],
                        at[:, k, m * 128 : (m + 1) * 128],
                        wk_lo[:, k, :],
                        start=(k == 0),
                        stop=(k == KS - 1),
                    )
            for m in range(M):
                nc.scalar.activation(sig_if[:, m, :], plo[m][:], Sig)
            # f * c_tm1 off the epilogue critical path (DVE is idle here)
            for m in range(M):
                nc.vector.tensor_mul(
                    fc_all[:, m, :], sig_if[:, m, UC : 2 * UC], ct[:, m, :]
                )

            # phase 2: per-m serial g|o accumulation + epilogue
            for m in range(M):
                ms = slice(m * 128, (m + 1) * 128)
                last = m == M - 1
                och = sb.tile([128, 512], bf16, tag="och", bufs=3)
                if not last:
                    phi = psum.tile([128, 512], f32, tag="ps", name=f"phi{m}")
                    for k in range(KS):
                        nc.tensor.matmul(
                            phi[:],
                            at[:, k, ms],
                            wk_hi[:, k, :],
                            start=(k == 0),
                            stop=(k == KS - 1),
                        )
                    phi_g, phi_o = phi[:, 0:UC], phi[:, UC : 2 * UC]
                else:
                    # split the last m so tanh(g)/i*g/c/tanh(c) overlap the
                    # o-half matmuls, shortening the end-of-kernel chain.
                    pg = psum.tile([128, UC], f32, tag="ps", name="pg")
                    po = psum.tile([128, UC], f32, tag="ps", name="po")
                    for k in range(KS):
                        nc.tensor.matmul(
                            pg[:],
                            at[:, k, ms],
                            wk_hi[:, k, 0:UC],
                            start=(k == 0),
                            stop=(k == KS - 1),
                        )
                    for k in range(KS):
                        nc.tensor.matmul(
                            po[:],
                            at[:, k, ms],
                            wk_hi[:, k, UC : 2 * UC],
                            start=(k == 0),
                            stop=(k == KS - 1),
                        )
                    phi_g, phi_o = pg[:], po[:]
                tg = sb.tile([128, UC], bf16, tag="tg", bufs=3)
                nc.scalar.activation(tg[:], phi_g, Tanh)
                ig = sb.tile([128, UC], bf16, tag="ig", bufs=3)
                nc.vector.tensor_mul(ig[:], sig_if[:, m, 0:UC], tg[:])
                nc.vector.tensor_add(och[:, 0:UC], fc_all[:, m, :], ig[:])
                th = sb.tile([128, UC], bf16, tag="th", bufs=3)
                nc.scalar.activation(th[:], och[:, 0:UC], Tanh)
                so = sb.tile([128, UC], bf16, tag="so", bufs=3)
                nc.scalar.activation(so[:], phi_o, Sig)
                nc.vector.tensor_mul(och[:, UC : 2 * UC], so[:], th[:])
                # Sync HWDGE ring is idle during phase 2.
                if last:
                    nc.sync.dma_start(ch_out[ms, 0:UC], och[:, 0:UC])
                    nc.sync.dma_start(ch_out[ms, UC : 2 * UC], och[:, UC : 2 * UC])
                else:
                    nc.sync.dma_start(ch_out[ms, :], och[:])

    nc.compile()
    return nc


def get_nc():
    if "nc" not in _CACHE:
        _CACHE["nc"] = _build_nc()
    return _CACHE["nc"]


def make_in_maps(inputs, h_tm1, c_tm1, kernel):
    x = np.asarray(inputs, dtype=np.float32)
    h = np.asarray(h_tm1, dtype=np.float32)
    c = np.asarray(c_tm1, dtype=np.float32).astype(BF16)
    w = np.asarray(kernel, dtype=np.float32).astype(BF16)
    at_full = np.ascontiguousarray(
        np.concatenate([h, x], axis=1).T.astype(BF16)
    )  # [K, B] bf16
    in_maps = []
    for core in range(R * C):
        r, ci = divmod(core, C)
        at_np = np.ascontiguousarray(at_full[:, r * BR : (r + 1) * BR])
        gates = [
            w[:, g * UNITS + ci * UC : g * UNITS + (ci + 1) * UC] for g in range(4)
        ]
        wklo_np = np.ascontiguousarray(np.concatenate(gates[0:2], axis=1))
        wkhi_np = np.ascontiguousarray(np.concatenate(gates[2:4], axis=1))
        ct_np = np.ascontiguousarray(c[r * BR : (r + 1) * BR, ci * UC : (ci + 1) * UC])
        in_maps.append(
            {"at": at_np, "wk_lo": wklo_np, "wk_hi": wkhi_np, "ct": ct_np}
        )
    return in_maps


def assemble(results):
    h_new = np.empty((BATCH, UNITS), dtype=np.float32)
    c_new = np.empty((BATCH, UNITS), dtype=np.float32)
    for core in range(R * C):
        r, ci = divmod(core, C)
        ch = results[core]["ch_out"].astype(np.float32)
        c_new[r * BR : (r + 1) * BR, ci * UC : (ci + 1) * UC] = ch[:, 0:UC]
        h_new[r * BR : (r + 1) * BR, ci * UC : (ci + 1) * UC] = ch[:, UC : 2 * UC]
    return h_new, c_new


def kernel(inputs, h_tm1, c_tm1, kernel):
    from concourse.bass_utils import run_bass_kernel_spmd

    nc = get_nc()
    in_maps = make_in_maps(inputs, h_tm1, c_tm1, kernel)
    res = run_bass_kernel_spmd(nc, in_maps, list(range(R * C)), trace=False)
    return assemble(res.results)
